# revision 19
# baseline (speedup 1.0000x reference)
"""Multi-head attention kernel for Trainium2 (8 NeuronCores, data-parallel over batch).

v5: 3-term compensated fp8 DoubleRow projections (see below) + restructured
schedule that eliminates the DMA-serialized head and the long tail:

 - One merged DMA per input kind (e.g. all 4 d-chunk-pair tiles of xqh in a
   single 3D-AP transfer) so Q/K chunk-0 projection starts ~6us in and the
   first exp fires ~11us in (was 45us: 32 serial HWDGE transfers).
 - V_ext phase runs inside the attention-group body (4 of 16 psum groups per
   group window) instead of as a serial prologue.
 - AV lags scores by ONE head pair, so after the last exp only AV(pair 7)
   and the output projection remain.

Numerics: Q/K/V projections are error-compensated 3-term fp8e4 DoubleRow
(out = xh@wh + xl@wh + xh@wl, host-split hi/lo, weights pre-scaled by 32 so
their residuals stay out of e4m3's subnormal range). DoubleRow = 256-deep
contraction at 0.5 cycles/row -> 49152 PE cycles per projection vs 65536
bf16. Scores/AV/out-proj stay bf16: fp8 there costs 2-3.5% max-rel error
(measured) vs the 2e-2 gate, and 3-term compensation is not cheaper than
bf16 on a 64-deep contraction.

Scale bookkeeping: qt/kt hold 32*(Q|K) in bf16 -> exp scale = SCALE/1024;
vext holds 32*V with its ones column memset to 32.0, so the softmax-rowsum
divide cancels the scale exactly.
"""

from contextlib import ExitStack

import numpy as np

import concourse.bass as bass
import concourse.mybir as mybir
import concourse.tile as tile
from concourse import bacc
from concourse.bass_utils import run_bass_kernel_spmd

F32 = mybir.dt.float32
BF = mybir.dt.bfloat16
FP8 = mybir.dt.float8e4
DR = mybir.MatmulPerfMode.DoubleRow
ALU = mybir.AluOpType
ACTF = mybir.ActivationFunctionType

B, T, D, H = 8, 1024, 1024, 16
HD = D // H
SCALE = HD**-0.5
WS = 32.0  # host-side weight pre-scale (power of 2)
ESCALE = SCALE / (WS * WS)
P = 128
PT = D // P  # 8 chunks
HE = HD + 1  # 65
DE = H * HE  # 1040


def _build(esc_bufs=24, av_bufs=2, sc_bufs=2, pj_bufs=2, qt_bufs=3, ysb_bufs=3):
    nc = bacc.Bacc(None, target_bir_lowering=False)
    dr_ins = {}
    for nm in ("xqh", "xql", "xkh", "xkl", "xvh", "xvl"):
        dr_ins[nm] = nc.dram_tensor(nm, [4, P, 2 * T], FP8, kind="ExternalInput")
    for nm in ("wqh", "wql", "wkh", "wkl", "wvh", "wvl"):
        dr_ins[nm] = nc.dram_tensor(nm, [4, P, 2 * D], FP8, kind="ExternalInput")
    wot_d = nc.dram_tensor("wot", [D, D], BF, kind="ExternalInput")
    bq_d = nc.dram_tensor("bq", [D], F32, kind="ExternalInput")  # 32*bq
    bk_d = nc.dram_tensor("bk", [D], F32, kind="ExternalInput")  # 32*bk
    bvh_d = nc.dram_tensor("bvh", [D], BF, kind="ExternalInput")  # 32*bv
    boh_d = nc.dram_tensor("boh", [D], BF, kind="ExternalInput")
    y_d = nc.dram_tensor("y", [T, D], F32, kind="ExternalOutput")

    with tile.TileContext(nc) as tc, ExitStack() as top:
        consts = top.enter_context(tc.tile_pool(name="consts", bufs=1, side="left"))
        bqT = consts.tile([P, PT], F32, tag="bqT")
        nc.gpsimd.dma_start(out=bqT, in_=bq_d[:].rearrange("(k p) -> p k", p=P))
        bkT = consts.tile([P, PT], F32, tag="bkT")
        nc.gpsimd.dma_start(out=bkT, in_=bk_d[:].rearrange("(k p) -> p k", p=P))
        bvb = consts.tile([P, D], BF, tag="bvb")
        nc.gpsimd.dma_start(
            out=bvb, in_=bass.AP(tensor=bvh_d, offset=0, ap=[[0, P], [1, D]])
        )
        bob = consts.tile([P, D], BF, tag="bob")
        nc.gpsimd.dma_start(
            out=bob, in_=bass.AP(tensor=boh_d, offset=0, ap=[[0, P], [1, D]])
        )

        ident = consts.tile([P, P], BF, tag="ident")
        from concourse.masks import make_identity

        make_identity(nc, ident)

        # persistent left pools
        vext_pool = top.enter_context(tc.tile_pool(name="vext", bufs=PT, side="left"))
        vext = [
            vext_pool.tile([P, DE], BF, tag="vext", name=f"vext{i}") for i in range(PT)
        ]
        for k in range(PT):
            # ones column at 32.0 cancels the 32x V scale in the rowsum divide
            nc.gpsimd.memset(
                vext[k].rearrange("p (h x) -> p h x", x=HE)[:, :, HD:HE], WS
            )
        otb_pool = top.enter_context(tc.tile_pool(name="otb", bufs=1, side="left"))
        otb = otb_pool.tile([P, PT * T], BF, tag="otb", name="otb")
        otb3 = otb.rearrange("p (k t) -> p k t", t=T)

        # streaming pools (right side)
        qkp = top.enter_context(tc.tile_pool(name="qkp", bufs=8, side="right"))
        vwp = top.enter_context(tc.tile_pool(name="vwp", bufs=4, side="right"))
        qt_pool = top.enter_context(tc.tile_pool(name="qt", bufs=qt_bufs, side="right"))
        kt_pool = top.enter_context(tc.tile_pool(name="kt", bufs=qt_bufs, side="right"))
        esc_pool = top.enter_context(
            tc.tile_pool(name="esc", bufs=esc_bufs, side="right")
        )
        obq_pool = top.enter_context(tc.tile_pool(name="obq", bufs=2, side="right"))
        smalls = top.enter_context(tc.tile_pool(name="smalls", bufs=1, side="right"))
        ps = top.enter_context(tc.tile_pool(name="ps", bufs=1, space="PSUM"))

        # ---- merged input DMA: one transfer per kind ----
        # sync queue: Q then K kinds (first projection starts after 4 DMAs);
        # gpsimd queue: V kinds, then wo (reusing V slots).
        def big_in(pool, nm, tag):
            dram = dr_ins[nm]
            nf = dram.shape[2]
            tt = pool.tile([P, 4 * nf], FP8, tag=tag, name=nm)
            src = bass.AP(tensor=dram, offset=0, ap=[[nf, P], [P * nf, 4], [1, nf]])
            if tag == "qk":
                nc.sync.dma_start(out=tt, in_=src)
            else:
                nc.gpsimd.dma_start(out=tt, in_=src)
            r = tt.rearrange("p (c two t) -> p c two t", c=4, two=2)
            return [r[:, c2] for c2 in range(4)]

        qk = {}
        for nm in ("wqh", "xqh", "wql", "xql", "wkh", "xkh", "wkl", "xkl"):
            qk[nm] = big_in(qkp, nm, "qk")
        xvh = big_in(vwp, "xvh", "vw")
        wvh = big_in(vwp, "wvh", "vw")
        xvl = big_in(vwp, "xvl", "vw")
        wvl = big_in(vwp, "wvl", "vw")
        # wo: two 8KB tiles rotating into the V slots after the V phase
        wo = []
        for half in range(2):
            wt = vwp.tile([P, 4 * T], BF, tag="vw", name=f"wo{half}")
            nc.gpsimd.dma_start(
                out=wt,
                in_=bass.AP(
                    tensor=wot_d,
                    offset=half * 512 * D,
                    ap=[[D, P], [P * D, 4], [1, D]],
                ),
            )
            wor = wt.rearrange("p (k t) -> p k t", k=4)
            wo.extend([wor[:, j] for j in range(4)])

        # ---- item factories ----

        def v_items():
            """16 items: (s-block k, i-half ci) 3-term fp8 V_ext groups."""
            items = []

            def mk(k, ci):
                def run():
                    pv = ps.tile([P, 512], F32, tag="pj", bufs=pj_bufs, name=f"pv{k}_{ci}")
                    for c2 in range(4):
                        for ti, (xs, ws) in enumerate(
                            ((xvh, wvh), (xvl, wvh), (xvh, wvl))
                        ):
                            nc.tensor.matmul(
                                pv[:, :],
                                xs[c2][:, :, 128 * k : 128 * (k + 1)],
                                ws[c2][:, :, 512 * ci : 512 * (ci + 1)],
                                start=(c2 == 0 and ti == 0),
                                stop=(c2 == 3 and ti == 2),
                                perf_mode=DR,
                            )
                    nc.vector.tensor_tensor(
                        out=vext[k].rearrange("p (h x) -> p h x", x=HE)[
                            :, 8 * ci : 8 * (ci + 1), 0:HD
                        ],
                        in0=pv.rearrange("p (h x) -> p h x", x=HD),
                        in1=bvb[:, 512 * ci : 512 * (ci + 1)].rearrange(
                            "p (h x) -> p h x", x=HD
                        ),
                        op=ALU.add,
                    )

                return run

            for k in range(PT):
                for ci in range(2):
                    items.append(mk(k, ci))
            return items

        qt = {}
        kt = {}

        def proj_items(dst, k, wh, wl, xh, xl, bias):
            """4 closures; each emits 6 of the 12 DoubleRow instrs of one
            512-col projection (c2-pairs 01 / 23)."""
            items = []
            state = {}

            def mk(c, half):
                def run():
                    if half == 0:
                        state[c] = ps.tile([P, 512], F32, tag="pj", bufs=pj_bufs, name=f"pj{k}_{c}")
                    pt_ = state[c]
                    for c2 in range(2 * half, 2 * half + 2):
                        for ti, (ws_, xs_) in enumerate(
                            ((wh, xh), (wh, xl), (wl, xh))
                        ):
                            nc.tensor.matmul(
                                pt_[:, :],
                                ws_[c2][:, :, 128 * k : 128 * (k + 1)],
                                xs_[c2][:, :, 512 * c : 512 * (c + 1)],
                                start=(c2 == 0 and ti == 0),
                                stop=(c2 == 3 and ti == 2),
                                perf_mode=DR,
                            )
                    if half == 1:
                        nc.vector.tensor_scalar(
                            out=dst[:, 512 * c : 512 * (c + 1)],
                            in0=pt_[:, :],
                            scalar1=bias[:, k : k + 1],
                            scalar2=None,
                            op0=ALU.add,
                        )

                return run

            for c in range(2):
                items.append(mk(c, 0))
                items.append(mk(c, 1))
            return items

        def make_qk_items(k):
            qt[k] = qt_pool.tile([P, T], BF, tag="qt", name=f"qt{k}")
            kt[k] = kt_pool.tile([P, T], BF, tag="kt", name=f"kt{k}")
            return proj_items(
                qt[k], k, qk["wqh"], qk["wql"], qk["xqh"], qk["xql"], bqT
            ) + proj_items(
                kt[k], k, qk["wkh"], qk["wkl"], qk["xkh"], qk["xkl"], bkT
            )

        esc = {}

        def make_sc_items(h):
            hi, ro = h // 2, 64 * (h % 2)
            esc[h] = []

            def mk(s):
                def run():
                    psc = ps.tile([P, T], F32, tag="sc", bufs=sc_bufs, name=f"sc{h}_{s}")
                    for c in range(2):
                        nc.tensor.matmul(
                            psc[:, 512 * c : 512 * (c + 1)],
                            kt[hi][ro : ro + 64, 128 * s : 128 * (s + 1)],
                            qt[hi][ro : ro + 64, 512 * c : 512 * (c + 1)],
                            start=True,
                            stop=True,
                        )
                    e = esc_pool.tile([P, T], BF, tag="esc", name=f"esc{h}_{s}")
                    nc.scalar.activation(out=e, in_=psc[:, :], func=ACTF.Exp, scale=ESCALE)
                    esc[h].append(e)

                return run

            return [mk(s) for s in range(PT)]

        obq = {}

        def make_av_items(h):
            q = h // 4
            if q not in obq:
                t_ = obq_pool.tile([P, PT * 256], BF, tag="ob", bufs=2, name=f"ob{q}")
                obq[q] = t_.rearrange("p (t i) -> p t i", i=256)
            ob = obq[q]
            col = 64 * (h % 4)

            def mk(tm):
                def run():
                    pav = ps.tile([P, HE], F32, tag="av", bufs=av_bufs, name=f"av{h}_{tm}")
                    for s in range(PT):
                        nc.tensor.matmul(
                            pav[:, :],
                            esc[h][s][:, 128 * tm : 128 * (tm + 1)],
                            vext[s][:, HE * h : HE * (h + 1)],
                            start=(s == 0),
                            stop=(s == PT - 1),
                            skip_group_check=True,
                        )
                    rcp = smalls.tile([P, 1], F32, tag="rcp", bufs=6, name=f"rcp{h}_{tm}")
                    nc.vector.reciprocal(rcp, pav[:, HD : HD + 1])
                    nc.vector.tensor_scalar(
                        out=ob[:, tm, col : col + HD],
                        in0=pav[:, 0:HD],
                        scalar1=rcp,
                        scalar2=None,
                        op0=ALU.mult,
                    )
                    if h % 2 == 1:
                        p_ = h // 2
                        if h == H - 1:
                            # final pair: transpose on PE + DVE evict (no xbar
                            # DMA latency on the critical tail)
                            tps = tail_ps[:, 64 * tm : 64 * (tm + 1)].bitcast(BF)
                            nc.tensor.transpose(
                                tps,
                                ob[:, tm, 128 * (p_ % 2) : 128 * (p_ % 2) + 128],
                                ident,
                            )
                            nc.vector.tensor_copy(
                                otb3[:, p_, 128 * tm : 128 * (tm + 1)], tps
                            )
                        else:
                            nc.sync.dma_start_transpose(
                                out=otb3[:, p_, 128 * tm : 128 * (tm + 1)],
                                in_=ob[:, tm, 128 * (p_ % 2) : 128 * (p_ % 2) + 128],
                            )

                return run

            return [mk(tm) for tm in range(PT)]

        # ---- output projection helpers (2-wave) ----
        # wave A (k<4) runs inside group 7's ACT window, accumulating into
        # SBUF tiles that reuse freed qk-input slots; wave B (k>=4) runs at
        # the tail interleaved with AV pair 7.
        yacc = {}

        def wavea_items():
            items = []

            def mk(c, m):
                def run():
                    j = (2 * m + c) // 4  # 4 acc tiles of 4 chunks each
                    if j not in yacc:
                        t_ = qkp.tile([P, 2 * T], F32, tag="qk", name=f"yacc{j}")
                        yacc[j] = t_.rearrange("p (s f) -> p s f", f=512)
                    psy = ps.tile([P, 512], F32, tag="pj", bufs=pj_bufs, name=f"pya{c}_{m}")
                    for k in range(4):
                        nc.tensor.matmul(
                            psy,
                            otb3[:, k, 128 * m : 128 * (m + 1)],
                            wo[k][:, 512 * c : 512 * (c + 1)],
                            start=(k == 0),
                            stop=(k == 3),
                        )
                    nc.vector.tensor_tensor(
                        out=yacc[j][:, (2 * m + c) % 4, :],
                        in0=psy,
                        in1=bob[:, 512 * c : 512 * (c + 1)],
                        op=ALU.add,
                    )

                return run

            for m in range(PT):
                for c in range(2):
                    items.append(mk(c, m))
            return items

        def waveb_item(c, m):
            def run():
                if m % 2 == 0:
                    psy = ps.tile([P, 512], F32, tag="pj", bufs=pj_bufs, name=f"pyb{c}_{m}")
                else:
                    pyt = ps.tile([P, T], F32, tag="sc", bufs=sc_bufs, name=f"pyb{c}_{m}")
                    psy = pyt[:, 0:512]
                for k in range(4, PT):
                    nc.tensor.matmul(
                        psy,
                        otb3[:, k, 128 * m : 128 * (m + 1)],
                        wo[k][:, 512 * c : 512 * (c + 1)],
                        start=(k == 4),
                        stop=(k == PT - 1),
                    )
                j = (2 * m + c) // 4
                ysb = smalls.tile([P, 512], F32, tag="ysb", bufs=ysb_bufs, name=f"ysb{c}_{m}")
                nc.vector.tensor_tensor(
                    out=ysb,
                    in0=psy,
                    in1=yacc[j][:, (2 * m + c) % 4, :],
                    op=ALU.add,
                )
                nc.scalar.dma_start(
                    out=y_d[128 * m : 128 * (m + 1), 512 * c : 512 * (c + 1)],
                    in_=ysb,
                )

            return run

        # ---- schedule ----
        # pre-loop: QT(0)/KT(0) projections only (enabled ~6us in by the
        # merged Q-kind DMAs).
        vit = v_items()
        for it in make_qk_items(0):
            it()

        # groups: scores+exp for pair k; V_ext groups fill groups 0-1 (all
        # emitted before any AV reads vext); AV lags by TWO pairs from group
        # 2; projection for pair k+1; wave-A out-proj inside group 7.
        for k in range(PT):
            sc_items = make_sc_items(2 * k) + make_sc_items(2 * k + 1)
            qk_items = make_qk_items(k + 1) if k < PT - 1 else []
            av_items = (
                make_av_items(2 * (k - 2)) + make_av_items(2 * (k - 2) + 1)
                if k >= 2
                else []
            )
            vslice = vit[8 * k : 8 * (k + 1)] if k <= 1 else []
            wa_items = wavea_items() if k == PT - 1 else []
            for i in range(16):
                sc_items[i]()
                if av_items:
                    av_items[i]()
                if qk_items and i % 2 == 1:
                    qk_items[(i - 1) // 2]()
                if vslice and i % 2 == 0:
                    vslice[i // 2]()
                if wa_items:
                    wa_items[i]()

        # tail: AV pairs 6 and 7; pair 7's transposes run on the PE and its
        # per-tm completion releases the matching wave-B out-proj chunk.
        tail_ps = ps.tile([P, T], F32, tag="sc", bufs=sc_bufs, name="tail_ps")
        for it in make_av_items(12) + make_av_items(13):
            it()
        av7 = make_av_items(14) + make_av_items(15)
        for tm in range(PT):
            av7[tm]()
            av7[8 + tm]()
            waveb_item(0, tm)()
            waveb_item(1, tm)()

    nc.compile()
    return nc


_NC_CACHE = None


def _get_nc():
    global _NC_CACHE
    if _NC_CACHE is None:
        _NC_CACHE = _build()
    return _NC_CACHE


def _pairs(a):
    """[1024, n] -> [4, 128, 2n]: d-chunk pairs, k-halves along free dim."""
    n = a.shape[1]
    return np.ascontiguousarray(
        a.reshape(4, 2, 128, n).transpose(0, 2, 1, 3).reshape(4, 128, 2 * n)
    )


def kernel(**inputs) -> np.ndarray:
    import ml_dtypes

    bf16 = ml_dtypes.bfloat16
    e4m3 = ml_dtypes.float8_e4m3

    def split_pairs(a):
        hi = a.astype(e4m3)
        lo = (a - hi.astype(np.float32)).astype(e4m3)
        return _pairs(hi), _pairs(lo)

    query = np.asarray(inputs["query"], dtype=np.float32)
    key = np.asarray(inputs["key"], dtype=np.float32)
    value = np.asarray(inputs["value"], dtype=np.float32)

    wqh, wql = split_pairs(np.asarray(inputs["Wq"], np.float32).T * WS)
    wkh, wkl = split_pairs(np.asarray(inputs["Wk"], np.float32).T * WS)
    wvh, wvl = split_pairs(np.asarray(inputs["Wv"], np.float32).T * WS)
    wot = np.ascontiguousarray(np.asarray(inputs["Wo"], np.float32).T).astype(bf16)

    bq = np.ascontiguousarray(np.asarray(inputs["bq"], np.float32) * WS)
    bk = np.ascontiguousarray(np.asarray(inputs["bk"], np.float32) * WS)
    bvh = (np.asarray(inputs["bv"], np.float32) * WS).astype(bf16)
    boh = np.asarray(inputs["bo"], np.float32).astype(bf16)

    nc = _get_nc()
    in_maps = []
    for b in range(B):
        xqh, xql = split_pairs(np.ascontiguousarray(query[b].T))
        xkh, xkl = split_pairs(np.ascontiguousarray(key[b].T))
        xvh, xvl = split_pairs(np.ascontiguousarray(value[b].T))
        in_maps.append(
            {
                "xqh": xqh, "xql": xql,
                "xkh": xkh, "xkl": xkl,
                "xvh": xvh, "xvl": xvl,
                "wqh": wqh, "wql": wql,
                "wkh": wkh, "wkl": wkl,
                "wvh": wvh, "wvl": wvl,
                "wot": wot,
                "bq": bq, "bk": bk, "bvh": bvh, "boh": boh,
            }
        )
    res = run_bass_kernel_spmd(nc, in_maps, core_ids=list(range(B)))
    return np.stack([res.results[b]["y"] for b in range(B)], axis=0)


# revision 21
# speedup vs baseline: 1.0387x; 1.0387x over previous
"""Multi-head attention kernel for Trainium2 (8 NeuronCores, data-parallel over batch).

v5: 3-term compensated fp8 DoubleRow projections (see below) + restructured
schedule that eliminates the DMA-serialized head and the long tail:

 - One merged DMA per input kind (e.g. all 4 d-chunk-pair tiles of xqh in a
   single 3D-AP transfer) so Q/K chunk-0 projection starts ~6us in and the
   first exp fires ~11us in (was 45us: 32 serial HWDGE transfers).
 - V_ext phase runs inside the attention-group body (4 of 16 psum groups per
   group window) instead of as a serial prologue.
 - AV lags scores by ONE head pair, so after the last exp only AV(pair 7)
   and the output projection remain.

Numerics: Q/K/V projections are error-compensated 3-term fp8e4 DoubleRow
(out = xh@wh + xl@wh + xh@wl, host-split hi/lo, weights pre-scaled by 32 so
their residuals stay out of e4m3's subnormal range). DoubleRow = 256-deep
contraction at 0.5 cycles/row -> 49152 PE cycles per projection vs 65536
bf16. Scores/AV/out-proj stay bf16: fp8 there costs 2-3.5% max-rel error
(measured) vs the 2e-2 gate, and 3-term compensation is not cheaper than
bf16 on a 64-deep contraction.

Scale bookkeeping: qt/kt hold 32*(Q|K) in bf16 -> exp scale = SCALE/1024;
vext holds 32*V with its ones column memset to 32.0, so the softmax-rowsum
divide cancels the scale exactly.
"""

from contextlib import ExitStack

import numpy as np

import concourse.bass as bass
import concourse.mybir as mybir
import concourse.tile as tile
from concourse import bacc
from concourse.bass_utils import run_bass_kernel_spmd

F32 = mybir.dt.float32
BF = mybir.dt.bfloat16
FP8 = mybir.dt.float8e4
DR = mybir.MatmulPerfMode.DoubleRow
ALU = mybir.AluOpType
ACTF = mybir.ActivationFunctionType

B, T, D, H = 8, 1024, 1024, 16
HD = D // H
SCALE = HD**-0.5
WS = 32.0  # host-side weight pre-scale (power of 2)
ESCALE = SCALE / (WS * WS)
P = 128
PT = D // P  # 8 chunks
HE = HD + 1  # 65
DE = H * HE  # 1040


def _build(esc_bufs=24, av_bufs=2, sc_bufs=2, pj_bufs=2, qt_bufs=3, ysb_bufs=3):
    nc = bacc.Bacc(None, target_bir_lowering=False)
    dr_ins = {}
    for nm in ("xqh", "xql", "xkh", "xkl", "xvh", "xvl"):
        dr_ins[nm] = nc.dram_tensor(nm, [4, P, 2 * T], FP8, kind="ExternalInput")
    for nm in ("wvh", "wvl"):
        dr_ins[nm] = nc.dram_tensor(nm, [4, P, 2 * D], FP8, kind="ExternalInput")
    for nm in ("wqh", "wql", "wkh", "wkl"):
        # column-block-major: [cb, p, (c2 kappa col)] so chunk-k weight slices
        # are 3D-contiguous DMAs
        dr_ins[nm] = nc.dram_tensor(nm, [PT, P, 8 * 128], FP8, kind="ExternalInput")
    wot_d = nc.dram_tensor("wot", [D, D], BF, kind="ExternalInput")
    bq_d = nc.dram_tensor("bq", [D], F32, kind="ExternalInput")  # 32*bq
    bk_d = nc.dram_tensor("bk", [D], F32, kind="ExternalInput")  # 32*bk
    bvh_d = nc.dram_tensor("bvh", [D], BF, kind="ExternalInput")  # 32*bv
    boh_d = nc.dram_tensor("boh", [D], BF, kind="ExternalInput")
    y_d = nc.dram_tensor("y", [T, D], F32, kind="ExternalOutput")

    with tile.TileContext(nc) as tc, ExitStack() as top:
        consts = top.enter_context(tc.tile_pool(name="consts", bufs=1, side="left"))
        bqT = consts.tile([P, PT], F32, tag="bqT")
        nc.gpsimd.dma_start(out=bqT, in_=bq_d[:].rearrange("(k p) -> p k", p=P))
        bkT = consts.tile([P, PT], F32, tag="bkT")
        nc.gpsimd.dma_start(out=bkT, in_=bk_d[:].rearrange("(k p) -> p k", p=P))
        bvb = consts.tile([P, D], BF, tag="bvb")
        nc.gpsimd.dma_start(
            out=bvb, in_=bass.AP(tensor=bvh_d, offset=0, ap=[[0, P], [1, D]])
        )
        bob = consts.tile([P, D], BF, tag="bob")
        nc.gpsimd.dma_start(
            out=bob, in_=bass.AP(tensor=boh_d, offset=0, ap=[[0, P], [1, D]])
        )

        ident = consts.tile([P, P], BF, tag="ident")
        from concourse.masks import make_identity

        make_identity(nc, ident)

        # persistent left pools
        vext_pool = top.enter_context(tc.tile_pool(name="vext", bufs=PT, side="left"))
        vext = [
            vext_pool.tile([P, DE], BF, tag="vext", name=f"vext{i}") for i in range(PT)
        ]
        for k in range(PT):
            # ones column at 32.0 cancels the 32x V scale in the rowsum divide
            nc.gpsimd.memset(
                vext[k].rearrange("p (h x) -> p h x", x=HE)[:, :, HD:HE], WS
            )
        otb_pool = top.enter_context(tc.tile_pool(name="otb", bufs=1, side="left"))
        otb = otb_pool.tile([P, PT * T], BF, tag="otb", name="otb")
        otb3 = otb.rearrange("p (k t) -> p k t", t=T)

        # streaming pools (right side)
        qkp = top.enter_context(tc.tile_pool(name="qkp", bufs=8, side="right"))
        vwp = top.enter_context(tc.tile_pool(name="vwp", bufs=4, side="right"))
        qt_pool = top.enter_context(tc.tile_pool(name="qt", bufs=qt_bufs, side="right"))
        kt_pool = top.enter_context(tc.tile_pool(name="kt", bufs=qt_bufs, side="right"))
        esc_pool = top.enter_context(
            tc.tile_pool(name="esc", bufs=esc_bufs, side="right")
        )
        obq_pool = top.enter_context(tc.tile_pool(name="obq", bufs=2, side="right"))
        smalls = top.enter_context(tc.tile_pool(name="smalls", bufs=1, side="right"))
        ps = top.enter_context(tc.tile_pool(name="ps", bufs=1, space="PSUM"))

        # ---- input DMA: consumption-ordered chunks on one queue ----
        # Transfers serialize on the DMA engines, so order IS the schedule:
        # Q/K w-blocks 0-1 -> Q/K x streams -> V streams -> w rest -> wo.
        def kind_tile(pool, nm, tag):
            dram = dr_ins[nm]
            nf = dram.shape[2]
            tt = pool.tile([P, 4 * nf] if dram.shape[0] == 4 else [P, PT * nf],
                           FP8, tag=tag, name=nm)
            return tt, dram, nf

        def chunk_dma(tt, dram, nf, c2, n=1):
            nc.sync.dma_start(
                out=tt[:, c2 * nf : (c2 + n) * nf],
                in_=bass.AP(
                    tensor=dram,
                    offset=c2 * P * nf,
                    ap=[[nf, P], [P * nf, n], [1, nf]],
                ),
            )

        kinds = {}
        for nm in ("wqh", "wql", "wkh", "wkl", "xqh", "xql", "xkh", "xkl"):
            kinds[nm] = kind_tile(qkp, nm, "qk")
        for nm in ("xvh", "xvl", "wvh", "wvl"):
            kinds[nm] = kind_tile(vwp, nm, "vw")

        # 1. Q/K w column-blocks 0-1 (projection chunks 0 and 1)
        for nm in ("wqh", "wql", "wkh", "wkl"):
            tt, dram, nf = kinds[nm]
            chunk_dma(tt, dram, nf, 0, n=2)
        # 2. full Q/K x streams
        for c2 in range(4):
            for nm in ("xqh", "xql", "xkh", "xkl"):
                tt, dram, nf = kinds[nm]
                chunk_dma(tt, dram, nf, c2)
        # 3. V streams
        for c2 in range(4):
            for nm in ("xvh", "xvl"):
                tt, dram, nf = kinds[nm]
                chunk_dma(tt, dram, nf, c2)
        for c2 in range(4):
            for nm in ("wvh", "wvl"):
                tt, dram, nf = kinds[nm]
                chunk_dma(tt, dram, nf, c2)
        # 4. Q/K w column-blocks 2-3, then 4-7
        for nm in ("wqh", "wql", "wkh", "wkl"):
            tt, dram, nf = kinds[nm]
            chunk_dma(tt, dram, nf, 2, n=2)
        for nm in ("wqh", "wql", "wkh", "wkl"):
            tt, dram, nf = kinds[nm]
            chunk_dma(tt, dram, nf, 4, n=4)

        def wview(nm):
            # [p, cb, c2, kappa, col]
            tt = kinds[nm][0]
            return tt.rearrange(
                "p (cb c2 two col) -> p cb c2 two col", cb=PT, c2=4, two=2
            )

        def xview(nm):
            tt = kinds[nm][0]
            r = tt.rearrange("p (c two t) -> p c two t", c=4, two=2)
            return [r[:, c2] for c2 in range(4)]

        qk = {nm: wview(nm) for nm in ("wqh", "wql", "wkh", "wkl")}
        qk.update({nm: xview(nm) for nm in ("xqh", "xql", "xkh", "xkl")})
        xvh = xview("xvh")
        xvl = xview("xvl")
        wvh = xview("wvh")
        wvl = xview("wvl")

        # 5. wo: two 8KB tiles rotating into the V slots after the V phase
        wo = []
        for half in range(2):
            wt = vwp.tile([P, 4 * T], BF, tag="vw", name=f"wo{half}")
            nc.sync.dma_start(
                out=wt,
                in_=bass.AP(
                    tensor=wot_d,
                    offset=half * 512 * D,
                    ap=[[D, P], [P * D, 4], [1, D]],
                ),
            )
            wor = wt.rearrange("p (k t) -> p k t", k=4)
            wo.extend([wor[:, j] for j in range(4)])

        # ---- item factories ----

        def v_items():
            """16 items: (s-block k, i-half ci) 3-term fp8 V_ext groups."""
            items = []

            def mk(k, ci):
                def run():
                    pv = ps.tile([P, 512], F32, tag="pj", bufs=pj_bufs, name=f"pv{k}_{ci}")
                    for c2 in range(4):
                        for ti, (xs, ws) in enumerate(
                            ((xvh, wvh), (xvl, wvh), (xvh, wvl))
                        ):
                            nc.tensor.matmul(
                                pv[:, :],
                                xs[c2][:, :, 128 * k : 128 * (k + 1)],
                                ws[c2][:, :, 512 * ci : 512 * (ci + 1)],
                                start=(c2 == 0 and ti == 0),
                                stop=(c2 == 3 and ti == 2),
                                perf_mode=DR,
                            )
                    nc.vector.tensor_tensor(
                        out=vext[k].rearrange("p (h x) -> p h x", x=HE)[
                            :, 8 * ci : 8 * (ci + 1), 0:HD
                        ],
                        in0=pv.rearrange("p (h x) -> p h x", x=HD),
                        in1=bvb[:, 512 * ci : 512 * (ci + 1)].rearrange(
                            "p (h x) -> p h x", x=HD
                        ),
                        op=ALU.add,
                    )

                return run

            for k in range(PT):
                for ci in range(2):
                    items.append(mk(k, ci))
            return items

        qt = {}
        kt = {}

        def proj_items(dst, k, wh, wl, xh, xl, bias):
            """4 closures; each emits 6 of the 12 DoubleRow instrs of one
            512-col projection (c2-pairs 01 / 23)."""
            items = []
            state = {}

            def mk(c, half):
                def run():
                    if half == 0:
                        state[c] = ps.tile([P, 512], F32, tag="pj", bufs=pj_bufs, name=f"pj{k}_{c}")
                    pt_ = state[c]
                    for c2 in range(2 * half, 2 * half + 2):
                        for ti, (ws_, xs_) in enumerate(
                            ((wh, xh), (wh, xl), (wl, xh))
                        ):
                            nc.tensor.matmul(
                                pt_[:, :],
                                ws_[:, k, c2],
                                xs_[c2][:, :, 512 * c : 512 * (c + 1)],
                                start=(c2 == 0 and ti == 0),
                                stop=(c2 == 3 and ti == 2),
                                perf_mode=DR,
                            )
                    if half == 1:
                        nc.vector.tensor_scalar(
                            out=dst[:, 512 * c : 512 * (c + 1)],
                            in0=pt_[:, :],
                            scalar1=bias[:, k : k + 1],
                            scalar2=None,
                            op0=ALU.add,
                        )

                return run

            for c in range(2):
                items.append(mk(c, 0))
                items.append(mk(c, 1))
            return items

        def make_qk_items(k):
            qt[k] = qt_pool.tile([P, T], BF, tag="qt", name=f"qt{k}")
            kt[k] = kt_pool.tile([P, T], BF, tag="kt", name=f"kt{k}")
            return proj_items(
                qt[k], k, qk["wqh"], qk["wql"], qk["xqh"], qk["xql"], bqT
            ) + proj_items(
                kt[k], k, qk["wkh"], qk["wkl"], qk["xkh"], qk["xkl"], bkT
            )

        esc = {}

        def make_sc_items(h):
            hi, ro = h // 2, 64 * (h % 2)
            esc[h] = []

            def mk(s):
                def run():
                    psc = ps.tile([P, T], F32, tag="sc", bufs=sc_bufs, name=f"sc{h}_{s}")
                    for c in range(2):
                        nc.tensor.matmul(
                            psc[:, 512 * c : 512 * (c + 1)],
                            kt[hi][ro : ro + 64, 128 * s : 128 * (s + 1)],
                            qt[hi][ro : ro + 64, 512 * c : 512 * (c + 1)],
                            start=True,
                            stop=True,
                        )
                    e = esc_pool.tile([P, T], BF, tag="esc", name=f"esc{h}_{s}")
                    nc.scalar.activation(out=e, in_=psc[:, :], func=ACTF.Exp, scale=ESCALE)
                    esc[h].append(e)

                return run

            return [mk(s) for s in range(PT)]

        obq = {}

        def make_av_items(h):
            q = h // 4
            if q not in obq:
                t_ = obq_pool.tile([P, PT * 256], BF, tag="ob", bufs=2, name=f"ob{q}")
                obq[q] = t_.rearrange("p (t i) -> p t i", i=256)
            ob = obq[q]
            col = 64 * (h % 4)

            def mk(tm):
                def run():
                    pav = ps.tile([P, HE], F32, tag="av", bufs=av_bufs, name=f"av{h}_{tm}")
                    for s in range(PT):
                        nc.tensor.matmul(
                            pav[:, :],
                            esc[h][s][:, 128 * tm : 128 * (tm + 1)],
                            vext[s][:, HE * h : HE * (h + 1)],
                            start=(s == 0),
                            stop=(s == PT - 1),
                            skip_group_check=True,
                        )
                    rcp = smalls.tile([P, 1], F32, tag="rcp", bufs=6, name=f"rcp{h}_{tm}")
                    nc.vector.reciprocal(rcp, pav[:, HD : HD + 1])
                    nc.vector.tensor_scalar(
                        out=ob[:, tm, col : col + HD],
                        in0=pav[:, 0:HD],
                        scalar1=rcp,
                        scalar2=None,
                        op0=ALU.mult,
                    )
                    if h % 2 == 1:
                        p_ = h // 2
                        if h == H - 1:
                            # final pair: transpose on PE + DVE evict (no xbar
                            # DMA latency on the critical tail)
                            tps = tail_ps[:, 64 * tm : 64 * (tm + 1)].bitcast(BF)
                            nc.tensor.transpose(
                                tps,
                                ob[:, tm, 128 * (p_ % 2) : 128 * (p_ % 2) + 128],
                                ident,
                            )
                            nc.vector.tensor_copy(
                                otb3[:, p_, 128 * tm : 128 * (tm + 1)], tps
                            )
                        else:
                            nc.sync.dma_start_transpose(
                                out=otb3[:, p_, 128 * tm : 128 * (tm + 1)],
                                in_=ob[:, tm, 128 * (p_ % 2) : 128 * (p_ % 2) + 128],
                            )

                return run

            return [mk(tm) for tm in range(PT)]

        # ---- output projection helpers (2-wave) ----
        # wave A (k<4) runs inside group 7's ACT window, accumulating into
        # SBUF tiles that reuse freed qk-input slots; wave B (k>=4) runs at
        # the tail interleaved with AV pair 7.
        yacc = {}

        def wavea_items():
            items = []

            def mk(c, m):
                def run():
                    j = (2 * m + c) // 4  # 4 acc tiles of 4 chunks each
                    if j not in yacc:
                        t_ = qkp.tile([P, 2 * T], F32, tag="qk", name=f"yacc{j}")
                        yacc[j] = t_.rearrange("p (s f) -> p s f", f=512)
                    psy = ps.tile([P, 512], F32, tag="pj", bufs=pj_bufs, name=f"pya{c}_{m}")
                    for k in range(4):
                        nc.tensor.matmul(
                            psy,
                            otb3[:, k, 128 * m : 128 * (m + 1)],
                            wo[k][:, 512 * c : 512 * (c + 1)],
                            start=(k == 0),
                            stop=(k == 3),
                        )
                    nc.vector.tensor_tensor(
                        out=yacc[j][:, (2 * m + c) % 4, :],
                        in0=psy,
                        in1=bob[:, 512 * c : 512 * (c + 1)],
                        op=ALU.add,
                    )

                return run

            for m in range(PT):
                for c in range(2):
                    items.append(mk(c, m))
            return items

        def waveb_item(c, m):
            def run():
                if m % 2 == 0:
                    psy = ps.tile([P, 512], F32, tag="pj", bufs=pj_bufs, name=f"pyb{c}_{m}")
                else:
                    pyt = ps.tile([P, T], F32, tag="sc", bufs=sc_bufs, name=f"pyb{c}_{m}")
                    psy = pyt[:, 0:512]
                for k in range(4, PT):
                    nc.tensor.matmul(
                        psy,
                        otb3[:, k, 128 * m : 128 * (m + 1)],
                        wo[k][:, 512 * c : 512 * (c + 1)],
                        start=(k == 4),
                        stop=(k == PT - 1),
                    )
                j = (2 * m + c) // 4
                ysb = smalls.tile([P, 512], F32, tag="ysb", bufs=ysb_bufs, name=f"ysb{c}_{m}")
                nc.vector.tensor_tensor(
                    out=ysb,
                    in0=psy,
                    in1=yacc[j][:, (2 * m + c) % 4, :],
                    op=ALU.add,
                )
                nc.scalar.dma_start(
                    out=y_d[128 * m : 128 * (m + 1), 512 * c : 512 * (c + 1)],
                    in_=ysb,
                )

            return run

        # ---- schedule ----
        # pre-loop: QT(0)/KT(0) projections only (enabled ~6us in by the
        # merged Q-kind DMAs).
        vit = v_items()
        for it in make_qk_items(0):
            it()

        # groups: scores+exp for pair k; V_ext groups fill groups 0-1 (all
        # emitted before any AV reads vext); AV lags by TWO pairs from group
        # 2; projection for pair k+1; wave-A out-proj inside group 7.
        av_sched = {3: (0, 1), 4: (2,), 5: (3,), 6: (4,), 7: (5,)}
        for k in range(PT):
            sc_items = make_sc_items(2 * k) + make_sc_items(2 * k + 1)
            qk_items = make_qk_items(k + 1) if k < PT - 1 else []
            av_items = []
            for pr in av_sched.get(k, ()):
                av_items += make_av_items(2 * pr) + make_av_items(2 * pr + 1)
            vslice = vit[8 * (k - 1) : 8 * k] if 1 <= k <= 2 else []
            wa_items = wavea_items() if k == PT - 1 else []
            for i in range(16):
                sc_items[i]()
                for av in av_items[i::16]:
                    av()
                if qk_items and i % 2 == 1:
                    qk_items[(i - 1) // 2]()
                if vslice and i % 2 == 0:
                    vslice[i // 2]()
                if wa_items:
                    wa_items[i]()

        # tail: AV pairs 6 and 7; pair 7's transposes run on the PE and its
        # per-tm completion releases the matching wave-B out-proj chunk.
        tail_ps = ps.tile([P, T], F32, tag="sc", bufs=sc_bufs, name="tail_ps")
        for it in make_av_items(12) + make_av_items(13):
            it()
        av7 = make_av_items(14) + make_av_items(15)
        for tm in range(PT):
            av7[tm]()
            av7[8 + tm]()
            waveb_item(0, tm)()
            waveb_item(1, tm)()

    nc.compile()
    return nc


_NC_CACHE = None


def _get_nc():
    global _NC_CACHE
    if _NC_CACHE is None:
        _NC_CACHE = _build()
    return _NC_CACHE


def _pairs(a):
    """[1024, n] -> [4, 128, 2n]: d-chunk pairs, k-halves along free dim."""
    n = a.shape[1]
    return np.ascontiguousarray(
        a.reshape(4, 2, 128, n).transpose(0, 2, 1, 3).reshape(4, 128, 2 * n)
    )


def kernel(**inputs) -> np.ndarray:
    import ml_dtypes

    bf16 = ml_dtypes.bfloat16
    e4m3 = ml_dtypes.float8_e4m3

    def split_pairs(a):
        hi = a.astype(e4m3)
        lo = (a - hi.astype(np.float32)).astype(e4m3)
        return _pairs(hi), _pairs(lo)

    query = np.asarray(inputs["query"], dtype=np.float32)
    key = np.asarray(inputs["key"], dtype=np.float32)
    value = np.asarray(inputs["value"], dtype=np.float32)

    def split_wcb(a):  # [d, i] -> hi, lo in [cb, p, (c2 kappa col)] layout
        hi = a.astype(e4m3)
        lo = (a - hi.astype(np.float32)).astype(e4m3)

        def pack(w):
            # [256*c2+128*kappa+p, 128*cb+col] -> [cb, p, c2, kappa, col]
            w5 = w.reshape(4, 2, 128, PT, 128).transpose(3, 2, 0, 1, 4)
            return np.ascontiguousarray(w5.reshape(PT, 128, 8 * 128))

        return pack(hi), pack(lo)

    wqh, wql = split_wcb(np.asarray(inputs["Wq"], np.float32).T * WS)
    wkh, wkl = split_wcb(np.asarray(inputs["Wk"], np.float32).T * WS)
    wvh, wvl = split_pairs(np.asarray(inputs["Wv"], np.float32).T * WS)
    wot = np.ascontiguousarray(np.asarray(inputs["Wo"], np.float32).T).astype(bf16)

    bq = np.ascontiguousarray(np.asarray(inputs["bq"], np.float32) * WS)
    bk = np.ascontiguousarray(np.asarray(inputs["bk"], np.float32) * WS)
    bvh = (np.asarray(inputs["bv"], np.float32) * WS).astype(bf16)
    boh = np.asarray(inputs["bo"], np.float32).astype(bf16)

    nc = _get_nc()
    in_maps = []
    for b in range(B):
        xqh, xql = split_pairs(np.ascontiguousarray(query[b].T))
        xkh, xkl = split_pairs(np.ascontiguousarray(key[b].T))
        xvh, xvl = split_pairs(np.ascontiguousarray(value[b].T))
        in_maps.append(
            {
                "xqh": xqh, "xql": xql,
                "xkh": xkh, "xkl": xkl,
                "xvh": xvh, "xvl": xvl,
                "wqh": wqh, "wql": wql,
                "wkh": wkh, "wkl": wkl,
                "wvh": wvh, "wvl": wvl,
                "wot": wot,
                "bq": bq, "bk": bk, "bvh": bvh, "boh": boh,
            }
        )
    res = run_bass_kernel_spmd(nc, in_maps, core_ids=list(range(B)))
    return np.stack([res.results[b]["y"] for b in range(B)], axis=0)
